# revision 1
# baseline (speedup 1.0000x reference)
"""Trainium2 Bass kernel for nn_CrossAttention (B=2, N=2048, D=1024, H=16).

Sharding (8 cores): core c -> (batch b = c//4, head-group hg = c%4).
Each head-group is 4 heads = 256 of the 1024 projection dims.

Per core:
  QT = (Wq_hg @ X_b^T)      [256, 2048]   (transposed projections)
  KT = (Wk_hg @ X_b^T)      [256, 2048]
  V  = (X_b @ Wv_hg^T)      [2048, 256]   (natural orientation, + ones col)
  per head h, q-tile: ST = KT_h_tile^T-style scores [tok_k, tok_q] on PE,
  exp on ScalarE (scale folded, no max subtraction: logits ~ N(0,1)),
  PV matmul with ones-augmented V gives x^T and softmax denominators,
  out_partial = x_hg @ Wo_hg^T + bo/4   [2048, 1024]
  ReduceScatter(add) over the 4 cores of the batch -> each core owns a
  disjoint 512-token slice of the final output; host concatenates.

All matmuls run as float32r (FP22 reduced precision, 1 cycle/row on PE).
"""

import numpy as np

B = 2
NT = 2048
D = 1024
HEADS = 16
DH = 64
NCORES = 8
CPB = 4  # cores per batch
HG = HEADS // NCORES * 2  # 4 heads per core
HGD = HG * DH  # 256 cols per core
GROUPS = [[0, 1, 2, 3], [4, 5, 6, 7]]
SCALE = DH ** -0.5

_patched = False


def _patch_tile_drain():
    """This container's walrus rejects >1 sync-wait on a Drain
    (CoreV3GenImpl setupSyncWait<CTRL_NO_STRUCT>: "Too many sync wait
    commands").  Split the final TileContext drain's waits across a chain
    of single-wait drains; semaphores are monotonic so sequential waits
    are equivalent to one multi-wait."""
    global _patched
    if _patched:
        return
    import concourse.tile as tile
    import concourse.mybir as mybir
    from concourse.vector_clock import ScopedClock

    _uid = [0]

    def _split_multiwaits(nc):
        # Walrus here allows only ONE sync-wait per instruction; hoist
        # extra waits onto single-wait NoOps inserted just before, on the
        # same engine (engine execution is serial, sems are monotonic).
        for f in nc.m.functions:
            for bb in f.blocks:
                il = bb.instructions
                i = 0
                while i < len(il):
                    inst = il[i]
                    si = inst.sync_info
                    if si is not None and len(si.on_wait) > 1:
                        waits = list(si.on_wait)
                        inst.sync_info = mybir.SyncInfo(
                            on_wait=[waits[-1]], on_update=list(si.on_update)
                        )
                        for w in waits[:-1]:
                            _uid[0] += 1
                            nop = mybir.InstEventSemaphore(
                                name=f"WSPLIT-{_uid[0]}",
                                engine=inst.engine,
                                ins=[],
                                outs=[],
                                sync_info=mybir.SyncInfo(
                                    on_wait=[w], on_update=[]),
                            )
                            il.insert(i, nop)
                            i += 1
                    i += 1

    def _drain_and_barrier(self, tick_clock, wait_clock):
        nc = self.nc
        drain_inst = nc.sync.drain()
        wait_clock.add_sem_waits(
            drain_inst.ins, ScopedClock({None: tick_clock.global_clock})
        )
        si = drain_inst.ins.sync_info
        if si is not None and len(si.on_wait) > 1:
            waits = list(si.on_wait)
            drain_inst.ins.sync_info = mybir.SyncInfo(
                on_wait=[waits[0]], on_update=list(si.on_update)
            )
            for w in waits[1:]:
                extra = nc.sync.drain()
                extra.ins.sync_info = mybir.SyncInfo(on_wait=[w], on_update=[])

        _split_multiwaits(nc)
        nc.all_engine_barrier()
        assert self.sems is not None
        popped = nc._tile_sem_poison_stack.pop()
        assert popped is self._sem_poison
        nc.clear_and_free_semaphores(list(self.sems.allocated().values()))
        nc.all_engine_barrier()

    tile.TileContext._drain_and_barrier = _drain_and_barrier
    _patched = True


def build_program(nt=NT):
    """Build the SPMD Bass program (one NeuronCore's view)."""
    _patch_tile_drain()
    import concourse.bass as bass
    import concourse.tile as tile
    import concourse.mybir as mybir

    f32 = mybir.dt.float32
    f32r = mybir.dt.float32r
    EXP = mybir.ActivationFunctionType.Exp

    NQT = nt // 512   # q tiles (rhs free dim 512)
    NKT = nt // 128   # k tiles (PE contraction dim 128)
    NMT = nt // 128   # token m-tiles
    QSL = nt // CPB   # output token slice per core

    nc = bass.Bass("TRN2", target_bir_lowering=False, debug=False,
                   num_devices=NCORES)

    xqT = nc.dram_tensor("xqT", [D, nt], f32r, kind="ExternalInput")
    xkT = nc.dram_tensor("xkT", [D, nt], f32r, kind="ExternalInput")
    xvT = nc.dram_tensor("xvT", [D, nt], f32r, kind="ExternalInput")
    wqT = nc.dram_tensor("wqT", [D, HGD], f32r, kind="ExternalInput")
    wkT = nc.dram_tensor("wkT", [D, HGD], f32r, kind="ExternalInput")
    wvT = nc.dram_tensor("wvT", [D, HGD], f32r, kind="ExternalInput")
    woT = nc.dram_tensor("woT", [HGD, D], f32r, kind="ExternalInput")
    bo4 = nc.dram_tensor("bo4", [D], f32, kind="ExternalInput")
    out = nc.dram_tensor("out", [QSL, D], f32, kind="ExternalOutput")

    partial = nc.dram_tensor("partial", [nt, D], f32)
    rsout = nc.dram_tensor("rsout", [QSL, D], f32)
    rbounce = nc.dram_tensor("rbounce", [16, 512], f32)

    with tile.TileContext(nc) as tc:
        from contextlib import ExitStack
        with ExitStack() as ctx:
            const = ctx.enter_context(tc.tile_pool(name="const", bufs=1))
            persist = ctx.enter_context(tc.tile_pool(name="persist", bufs=1))
            rhs_pool = ctx.enter_context(tc.tile_pool(name="rhs", bufs=4))
            pt_pool = ctx.enter_context(tc.tile_pool(name="pt", bufs=4))
            misc = ctx.enter_context(tc.tile_pool(name="misc", bufs=4))
            outsb = ctx.enter_context(tc.tile_pool(name="outsb", bufs=3))
            # PSUM: 8 banks of [128, 512]f32 total.  One shared 4-slot pool
            # for all plain matmul accumulators ("mm" tag), 2 slots for the
            # attention x^T accumulators, 2 for the output projection.
            st_ps = ctx.enter_context(
                tc.tile_pool(name="st_ps", bufs=4, space="PSUM"))
            xa_ps = ctx.enter_context(
                tc.tile_pool(name="xa_ps", bufs=2, space="PSUM"))
            op_ps = ctx.enter_context(
                tc.tile_pool(name="op_ps", bufs=2, space="PSUM"))

            # --- constants -------------------------------------------------
            wq_sb = const.tile([128, 8, HGD], f32r)   # [k-part, k-tile, col]
            wk_sb = const.tile([128, 8, HGD], f32r)
            wv_sb = const.tile([128, 8, HGD], f32r)
            wo_sb = const.tile([128, 2, D], f32r)     # [d-part, hg k-tile, odim]
            bias_sb = const.tile([128, D], f32)      # bo/4 broadcast over parts
            nc.sync.dma_start(out=wq_sb[:], in_=wqT[:].rearrange(
                "(t p) c -> p t c", p=128))
            nc.sync.dma_start(out=wk_sb[:], in_=wkT[:].rearrange(
                "(t p) c -> p t c", p=128))
            nc.sync.dma_start(out=wv_sb[:], in_=wvT[:].rearrange(
                "(t p) c -> p t c", p=128))
            nc.sync.dma_start(out=wo_sb[:], in_=woT[:].rearrange(
                "(t p) c -> p t c", p=128))
            nc.sync.dma_start(out=bias_sb[:],
                              in_=bo4[:].partition_broadcast(128))
            ones_sb = const.tile([128, 1], f32)
            nc.vector.memset(ones_sb[:], 1.0)

            # --- persistent activations -----------------------------------
            qt_sb = persist.tile([128, 2, nt], f32r)   # QT: [qcol%128, qcol//128, tok]
            kt_sb = persist.tile([128, 2, nt], f32r)
            v_sb = persist.tile([128, NMT, HG * (DH + 1)], f32r)  # + ones col
            xt_sb = persist.tile([128, 2, nt], f32r)   # x^T (normalized)

            # --- Q/K projections: out[qcol, tok] = sum_k W^T[k,qcol] X^T[k,tok]
            for (src, wsb, dst) in ((xqT, wq_sb, qt_sb), (xkT, wk_sb, kt_sb)):
                for n in range(NQT):
                    acc = [st_ps.tile([128, 512], f32, tag="mm", name="qkacc")
                           for _ in range(2)]
                    for k in range(8):
                        rhs = rhs_pool.tile([128, 512], f32r, tag="projrhs")
                        nc.sync.dma_start(
                            out=rhs[:],
                            in_=src[128 * k:128 * (k + 1),
                                    512 * n:512 * (n + 1)])
                        for m in range(2):
                            nc.tensor.matmul(
                                acc[m][:],
                                wsb[:, k, 128 * m:128 * (m + 1)],
                                rhs[:],
                                start=(k == 0), stop=(k == 7))
                    for m in range(2):
                        nc.vector.tensor_copy(
                            dst[:, m, 512 * n:512 * (n + 1)], acc[m][:])

            # --- V projection: out[tok, vcol] = sum_k X^T[k,tok] W^T[k,vcol]
            for mg in range(nt // 512):
                acc = [st_ps.tile([128, 512], f32, tag="mm", name="vacc")
                       for _ in range(4)]
                for k in range(8):
                    lhs = rhs_pool.tile([128, 512], f32r, tag="projrhs")
                    nc.sync.dma_start(
                        out=lhs[:],
                        in_=xvT[128 * k:128 * (k + 1),
                                512 * mg:512 * (mg + 1)])
                    for m in range(4):
                        nc.tensor.matmul(
                            acc[m][:, 0:HGD],
                            lhs[:, 128 * m:128 * (m + 1)],
                            wv_sb[:, k, :],
                            start=(k == 0), stop=(k == 7))
                for m in range(4):
                    mt = 4 * mg + m
                    for h in range(HG):
                        nc.vector.tensor_copy(
                            v_sb[:, mt, (DH + 1) * h:(DH + 1) * h + DH],
                            acc[m][:, DH * h:DH * (h + 1)])
                        nc.vector.tensor_copy(
                            v_sb[:, mt, (DH + 1) * h + DH:(DH + 1) * (h + 1)],
                            ones_sb[:])

            # --- attention + output projection, per q-tile ----------------
            for qt in range(NQT):
                qsl = slice(512 * qt, 512 * (qt + 1))
                for hp in range(2):
                    xa = [xa_ps.tile([DH + 1, 512], f32, tag="xa", name="xa")
                          for _ in range(2)]
                    for kt in range(NKT):
                        st = [st_ps.tile([128, 512], f32, tag="mm", name="st")
                              for _ in range(2)]
                        for j in range(2):
                            p0 = 64 * j
                            nc.tensor.matmul(
                                st[j][:],
                                kt_sb[p0:p0 + 64, hp,
                                      128 * kt:128 * (kt + 1)],
                                qt_sb[p0:p0 + 64, hp, qsl],
                                tile_position=(p0, 0))
                        for j in range(2):
                            h = 2 * hp + j
                            pt = pt_pool.tile([128, 512], f32r)
                            nc.scalar.activation(pt[:], st[j][:], EXP,
                                                 scale=SCALE)
                            nc.tensor.matmul(
                                xa[j][:],
                                v_sb[:, kt,
                                     (DH + 1) * h:(DH + 1) * (h + 1)
                                     ],
                                pt[:],
                                start=(kt == 0), stop=(kt == NKT - 1))
                    # normalize: x[d, tok] /= sums[tok] (sums in row DH)
                    for j in range(2):
                        rr = misc.tile([DH + 1, 512], f32, tag="rr")
                        nc.vector.reciprocal(rr[DH:DH + 1, :],
                                             xa[j][DH:DH + 1, :])
                        bc = misc.tile([DH, 512], f32, tag="bc")
                        rb = rbounce[(4 * (qt % 2) + 2 * hp + j)
                                     % 16:(4 * (qt % 2) + 2 * hp + j) % 16 + 1,
                                     :]
                        nc.sync.dma_start(out=rb, in_=rr[DH:DH + 1, :])
                        nc.sync.dma_start(out=bc[:],
                                          in_=rb.partition_broadcast(DH))
                        if j == 0:
                            nc.vector.tensor_mul(
                                xt_sb[0:DH, hp, qsl], xa[j][0:DH, :], bc[:])
                        else:
                            tm = misc.tile([DH, 512], f32r, tag="tm")
                            nc.vector.tensor_mul(tm[:], xa[j][0:DH, :], bc[:])
                            nc.sync.dma_start(out=xt_sb[DH:128, hp, qsl],
                                              in_=tm[:])

                # out-proj for this q-tile:
                # partial[t, o] = sum_d x^T[d, t] Wo^T[d, o]  (+ bo/4)
                for n in range(2):
                    osl = slice(512 * n, 512 * (n + 1))
                    for m in range(4):
                        tsl = slice(512 * qt + 128 * m,
                                    512 * qt + 128 * (m + 1))
                        acc = op_ps.tile([128, 512], f32, tag="op")
                        for k in range(2):
                            nc.tensor.matmul(
                                acc[:],
                                xt_sb[:, k, tsl],
                                wo_sb[:, k, osl],
                                start=(k == 0), stop=(k == 1))
                        ob = outsb.tile([128, 512], f32)
                        nc.vector.tensor_add(ob[:], acc[:], bias_sb[:, osl])
                        nc.sync.dma_start(out=partial[tsl, osl], in_=ob[:])

            # --- reduce-scatter over the 4 cores of this batch ------------
            cc = nc.gpsimd.collective_compute(
                "ReduceScatter",
                mybir.AluOpType.add,
                replica_groups=GROUPS,
                ins=[partial[:]],
                outs=[rsout[:]],
            )
            fin = nc.sync.dma_start(out=out[:], in_=rsout[:])
            tile.add_dep_helper(fin.ins, cc.ins, reason="out after RS")

    return nc


_CACHE = {}


def _get_program(nt=NT):
    if nt not in _CACHE:
        _CACHE[nt] = build_program(nt)
    return _CACHE[nt]


def make_in_maps(query, key, value, Wq, Wk, Wv, Wo, bo, nt=NT):
    """Host-side sharding: per-core input dicts."""
    query = np.asarray(query, dtype=np.float32)
    key = np.asarray(key, dtype=np.float32)
    value = np.asarray(value, dtype=np.float32)
    Wq = np.asarray(Wq, dtype=np.float32)
    Wk = np.asarray(Wk, dtype=np.float32)
    Wv = np.asarray(Wv, dtype=np.float32)
    Wo = np.asarray(Wo, dtype=np.float32)
    bo = np.asarray(bo, dtype=np.float32)

    xT = [np.ascontiguousarray(x.T) for x in
          (query[0], key[0], value[0], query[1], key[1], value[1])]
    bo4 = (bo * 0.25).astype(np.float32)
    in_maps = []
    for c in range(NCORES):
        b, hg = divmod(c, CPB)
        hsl = slice(HGD * hg, HGD * (hg + 1))
        in_maps.append({
            "xqT": xT[3 * b + 0],
            "xkT": xT[3 * b + 1],
            "xvT": xT[3 * b + 2],
            "wqT": np.ascontiguousarray(Wq[hsl, :].T),
            "wkT": np.ascontiguousarray(Wk[hsl, :].T),
            "wvT": np.ascontiguousarray(Wv[hsl, :].T),
            "woT": np.ascontiguousarray(Wo[:, hsl].T),
            "bo4": bo4,
        })
    return in_maps


def assemble(results, nt=NT):
    """Concatenate per-core disjoint token slices into [B, NT, D]."""
    out = np.empty((B, nt, D), dtype=np.float32)
    qsl = nt // CPB
    for c in range(NCORES):
        b, p = divmod(c, CPB)
        out[b, qsl * p:qsl * (p + 1), :] = results[c]["out"]
    return out


def run(query, key, value, Wq, Wk, Wv, Wo, bo, nt=NT, trace=False):
    from concourse.bass_utils import run_bass_kernel_spmd
    nc = _get_program(nt)
    in_maps = make_in_maps(query, key, value, Wq, Wk, Wv, Wo, bo, nt=nt)
    res = run_bass_kernel_spmd(nc, in_maps, core_ids=list(range(NCORES)),
                               trace=trace)
    return assemble(res.results, nt=nt), res


def kernel(query, key, value, qpos=None, kpos=None, Wq=None, Wk=None,
           Wv=None, Wo=None, bo=None):
    out, _ = run(query, key, value, Wq, Wk, Wv, Wo, bo)
    return out



# revision 6
# speedup vs baseline: 1.6786x; 1.6786x over previous
"""Trainium2 Bass kernel for nn_CrossAttention (B=2, N=2048, D=1024, H=16).

Sharding (8 cores): core c -> (batch b = c//4, head-group hg = c%4).
Each head-group is 4 heads = 256 of the 1024 projection dims.

v2 design (bf16 matmuls, pipelined attention, chunked overlapped RS):
  - All matmul inputs bf16 (host converts); PSUM accumulation fp32.
  - X^T staged fully resident in SBUF; all input DMAs issued upfront.
  - Scores for the 2 heads of a half-group go into ONE [128,1024] psum
    tile (2 banks) so exp is a single [128,1024] Activation instruction.
  - Inner kt loop software-pipelined: scores(kt+1) issued before PV(kt)
    so the PE never waits on the Activation engine.
  - Softmax denominators ride along as a ones-column in V (row 64 of the
    PV accumulator); normalization uses reciprocal_approx_fast (DVE) +
    DMA partition-broadcast; copies/bias-adds go to GpSimd (Pool).
  - Output projection per 512-token q-tile, ReduceScatter (bf16, add)
    per q-tile chunk overlapped with the next tile's attention. Core i
    of each 4-core group owns token strip 512*qt + 128*i.
"""

import numpy as np

B = 2
NT = 2048
D = 1024
HEADS = 16
DH = 64
NCORES = 8
CPB = 4  # cores per batch
HG = 4   # heads per core
HGD = HG * DH  # 256 cols per core
GROUPS = [[0, 1, 2, 3], [4, 5, 6, 7]]
SCALE = DH ** -0.5
NQT = 4     # q tiles of 512
NKT = 16    # kv tiles of 128
VW = DH + 1  # v columns per head incl. ones column

_patched = False


def _patch_tile_drain():
    """This container's walrus rejects >1 sync-wait on a Drain
    (CoreV3GenImpl setupSyncWait<CTRL_NO_STRUCT>: "Too many sync wait
    commands").  Split the final TileContext drain's waits across a chain
    of single-wait drains; semaphores are monotonic so sequential waits
    are equivalent to one multi-wait."""
    global _patched
    if _patched:
        return
    import concourse.tile as tile
    import concourse.mybir as mybir
    from concourse.vector_clock import ScopedClock

    _uid = [0]

    def _split_multiwaits(nc):
        for f in nc.m.functions:
            for bb in f.blocks:
                il = bb.instructions
                i = 0
                while i < len(il):
                    inst = il[i]
                    si = inst.sync_info
                    if si is not None and len(si.on_wait) > 1:
                        waits = list(si.on_wait)
                        inst.sync_info = mybir.SyncInfo(
                            on_wait=[waits[-1]], on_update=list(si.on_update)
                        )
                        for w in waits[:-1]:
                            _uid[0] += 1
                            nop = mybir.InstEventSemaphore(
                                name=f"WSPLIT-{_uid[0]}",
                                engine=inst.engine,
                                ins=[],
                                outs=[],
                                sync_info=mybir.SyncInfo(
                                    on_wait=[w], on_update=[]),
                            )
                            il.insert(i, nop)
                            i += 1
                    i += 1

    def _drain_and_barrier(self, tick_clock, wait_clock):
        nc = self.nc
        drain_inst = nc.sync.drain()
        wait_clock.add_sem_waits(
            drain_inst.ins, ScopedClock({None: tick_clock.global_clock})
        )
        si = drain_inst.ins.sync_info
        if si is not None and len(si.on_wait) > 1:
            waits = list(si.on_wait)
            drain_inst.ins.sync_info = mybir.SyncInfo(
                on_wait=[waits[0]], on_update=list(si.on_update)
            )
            for w in waits[1:]:
                extra = nc.sync.drain()
                extra.ins.sync_info = mybir.SyncInfo(on_wait=[w], on_update=[])

        _split_multiwaits(nc)
        nc.all_engine_barrier()
        assert self.sems is not None
        popped = nc._tile_sem_poison_stack.pop()
        assert popped is self._sem_poison
        nc.clear_and_free_semaphores(list(self.sems.allocated().values()))
        nc.all_engine_barrier()

    tile.TileContext._drain_and_barrier = _drain_and_barrier
    _patched = True


def build_program():
    _patch_tile_drain()
    import concourse.bass as bass
    import concourse.tile as tile
    import concourse.mybir as mybir

    f32 = mybir.dt.float32
    bf16 = mybir.dt.bfloat16
    EXP = mybir.ActivationFunctionType.Exp

    nc = bass.Bass("TRN2", target_bir_lowering=False, debug=False,
                   num_devices=NCORES)

    xqT = nc.dram_tensor("xqT", [D, NT], bf16, kind="ExternalInput")
    xkT = nc.dram_tensor("xkT", [D, NT], bf16, kind="ExternalInput")
    xvT = nc.dram_tensor("xvT", [D, NT], bf16, kind="ExternalInput")
    wqT = nc.dram_tensor("wqT", [D, HGD], bf16, kind="ExternalInput")
    wkT = nc.dram_tensor("wkT", [D, HGD], bf16, kind="ExternalInput")
    wvT = nc.dram_tensor("wvT", [D, HGD], bf16, kind="ExternalInput")
    woT = nc.dram_tensor("woT", [HGD, D], bf16, kind="ExternalInput")
    bo4 = nc.dram_tensor("bo4", [D], f32, kind="ExternalInput")
    out = nc.dram_tensor("out", [NQT, 128, D], f32, kind="ExternalOutput")

    partial = nc.dram_tensor("partial", [NT, D], bf16)
    rsout = nc.dram_tensor("rsout", [NQT, 128, D], bf16)
    rbounce = nc.dram_tensor("rbounce", [16, 512], f32)

    with tile.TileContext(nc) as tc:
        from contextlib import ExitStack
        with ExitStack() as ctx:
            const = ctx.enter_context(tc.tile_pool(name="const", bufs=1))
            persist = ctx.enter_context(tc.tile_pool(name="persist", bufs=1))
            pt_pool = ctx.enter_context(tc.tile_pool(name="pt", bufs=3))
            normp = ctx.enter_context(tc.tile_pool(name="normp", bufs=2))
            outsb = ctx.enter_context(tc.tile_pool(name="outsb", bufs=2))
            # PSUM: 8 banks of [128, 2KB]. st tiles are [128,1024]f32 =
            # 2 banks each (scores for 2 heads side by side; also reused
            # as projection accumulators). xa = attention x^T accums,
            # op = out-projection accumulators.
            st_ps = ctx.enter_context(
                tc.tile_pool(name="st_ps", bufs=2, space="PSUM"))
            xa_ps = ctx.enter_context(
                tc.tile_pool(name="xa_ps", bufs=2, space="PSUM"))
            op_ps = ctx.enter_context(
                tc.tile_pool(name="op_ps", bufs=2, space="PSUM"))

            # --- constants + all input DMAs upfront ------------------------
            wq_sb = const.tile([128, 8, HGD], bf16)  # [k-part, k-tile, col]
            wk_sb = const.tile([128, 8, HGD], bf16)
            wv_sb = const.tile([128, 8, HGD], bf16)
            wo_sb = const.tile([128, 2, D], bf16)    # [d-part, hg k-tile, od]
            bias_sb = const.tile([128, D], f32)      # bo/4 bcast over parts
            nc.sync.dma_start(out=wk_sb[:], in_=wkT[:].rearrange(
                "(t p) c -> p t c", p=128))
            nc.sync.dma_start(out=wv_sb[:], in_=wvT[:].rearrange(
                "(t p) c -> p t c", p=128))
            nc.sync.dma_start(out=wq_sb[:], in_=wqT[:].rearrange(
                "(t p) c -> p t c", p=128))
            nc.sync.dma_start(out=wo_sb[:], in_=woT[:].rearrange(
                "(t p) c -> p t c", p=128))
            nc.sync.dma_start(out=bias_sb[:],
                              in_=bo4[:].partition_broadcast(128))

            # resident X^T staging: [part, k-tile, token]
            xk_st = persist.tile([128, 8, NT], bf16)
            xv_st = persist.tile([128, 8, NT], bf16)
            xq_st = persist.tile([128, 8, NT], bf16)
            for n in range(4):
                sl = slice(512 * n, 512 * (n + 1))
                nc.sync.dma_start(out=xk_st[:, :, sl], in_=xkT[:, sl]
                                  .rearrange("(t p) c -> p t c", p=128))
            for n in range(4):
                sl = slice(512 * n, 512 * (n + 1))
                nc.sync.dma_start(out=xv_st[:, :, sl], in_=xvT[:, sl]
                                  .rearrange("(t p) c -> p t c", p=128))
            for n in range(4):
                sl = slice(512 * n, 512 * (n + 1))
                nc.sync.dma_start(out=xq_st[:, :, sl], in_=xqT[:, sl]
                                  .rearrange("(t p) c -> p t c", p=128))

            # --- persistent activations -----------------------------------
            qt_sb = persist.tile([128, 2, NT], bf16)  # [qcol%128, qcol//128, tok]
            kt_sb = persist.tile([128, 2, NT], bf16)
            v_sb = persist.tile([128, NKT, HG * VW], bf16)  # + ones cols
            xt_sb = persist.tile([128, 2, NT], bf16)  # normalized x^T

            nc.vector.memset(v_sb[:], 1.0)

            # --- K projection: KT[qcol, tok] = sum_k W^T[k,qcol] X^T[k,tok]
            def qk_proj(xst, wsb, dst):
                for n in range(NQT):
                    acc = st_ps.tile([128, 1024], f32, tag="st", name="qkacc")
                    for k in range(8):
                        for m in range(2):
                            nc.tensor.matmul(
                                acc[:, 512 * m:512 * (m + 1)],
                                wsb[:, k, 128 * m:128 * (m + 1)],
                                xst[:, k, 512 * n:512 * (n + 1)],
                                start=(k == 0), stop=(k == 7))
                    for m in range(2):
                        nc.scalar.copy(
                            dst[:, m, 512 * n:512 * (n + 1)],
                            acc[:, 512 * m:512 * (m + 1)])

            qk_proj(xk_st, wk_sb, kt_sb)

            # --- V projection: V[tok, vcol] = sum_k X^T[k,tok] W^T[k,vcol]
            for mg in range(4):
                acc = st_ps.tile([128, 1024], f32, tag="st", name="vacc")
                # m outer / k inner: regions m=0,1 (and 2,3) share a psum
                # bank; interleaving two open accumulation groups in one
                # bank corrupts results, so finish each region first.
                for m in range(4):
                    for k in range(8):
                        nc.tensor.matmul(
                            acc[:, 256 * m:256 * m + HGD],
                            xv_st[:, k, 512 * mg + 128 * m:
                                  512 * mg + 128 * (m + 1)],
                            wv_sb[:, k, :],
                            start=(k == 0), stop=(k == 7))
                for m in range(4):
                    mt = 4 * mg + m
                    # strided copy: drop into [128, HG, DH] slots of v_sb,
                    # leaving the ones columns (index DH) intact
                    nc.vector.tensor_copy(
                        v_sb[:, mt, :].rearrange("p (h w) -> p h w", w=VW)
                        [:, :, 0:DH],
                        acc[:, 256 * m:256 * (m + 1)].rearrange(
                            "p (h w) -> p h w", w=DH))

            # --- Q proj tile n, then attention for q-tile n ----------------
            partial_dmas = [[] for _ in range(NQT)]

            def attention(qt):
                qsl = slice(512 * qt, 512 * (qt + 1))
                for hp in range(2):
                    xa = [xa_ps.tile([DH + 1, 512], f32, tag="xa", name="xa")
                          for _ in range(2)]

                    def scores(kt):
                        st = st_ps.tile([128, 1024], f32, tag="st", name="st")
                        for j in range(2):
                            p0 = 64 * j
                            nc.tensor.matmul(
                                st[:, 512 * j:512 * (j + 1)],
                                kt_sb[p0:p0 + 64, hp,
                                      128 * kt:128 * (kt + 1)],
                                qt_sb[p0:p0 + 64, hp, qsl],
                                tile_position=(p0, 0))
                        return st

                    def exp(st):
                        pt = pt_pool.tile([128, 1024], bf16)
                        nc.scalar.activation(pt[:], st[:], EXP, scale=SCALE)
                        return pt

                    def pv(kt, pt):
                        for j in range(2):
                            h = 2 * hp + j
                            nc.tensor.matmul(
                                xa[j][:],
                                v_sb[:, kt, VW * h:VW * (h + 1)],
                                pt[:, 512 * j:512 * (j + 1)],
                                start=(kt == 0), stop=(kt == NKT - 1))

                    pt_prev = exp(scores(0))
                    for kt in range(1, NKT):
                        st = scores(kt)
                        pv(kt - 1, pt_prev)
                        pt_prev = exp(st)
                    pv(NKT - 1, pt_prev)

                    # normalize: x[d, tok] /= denom[tok] (denoms in row DH)
                    for j in range(2):
                        rc = normp.tile([DH + 1, 512], f32, tag=f"rc{j}")
                        nc.vector.reciprocal(
                            rc[DH:DH + 1, :], xa[j][DH:DH + 1, :])
                        ridx = 4 * (qt % 2) + 2 * hp + j
                        rb = rbounce[ridx:ridx + 1, :]
                        nc.sync.dma_start(out=rb, in_=rc[DH:DH + 1, :])
                        bc = normp.tile([DH, 512], f32, tag=f"bc{j}")
                        nc.sync.dma_start(out=bc[:],
                                          in_=rb.partition_broadcast(DH))
                        if j == 0:
                            nc.vector.tensor_mul(
                                xt_sb[0:DH, hp, qsl], xa[j][0:DH, :], bc[:])
                        else:
                            tm = normp.tile([DH, 512], bf16, tag="tm")
                            nc.vector.tensor_mul(tm[:], xa[j][0:DH, :], bc[:])
                            nc.sync.dma_start(out=xt_sb[DH:128, hp, qsl],
                                              in_=tm[:])

                # out-proj: partial[t, o] = sum_d x^T[d, t] Wo^T[d, o] + bo/4
                for n in range(2):
                    osl = slice(512 * n, 512 * (n + 1))
                    for m in range(4):
                        tsl = slice(512 * qt + 128 * m,
                                    512 * qt + 128 * (m + 1))
                        acc = op_ps.tile([128, 512], f32, tag="op")
                        for k in range(2):
                            nc.tensor.matmul(
                                acc[:],
                                xt_sb[:, k, tsl],
                                wo_sb[:, k, osl],
                                start=(k == 0), stop=(k == 1))
                        ob = outsb.tile([128, 512], bf16, tag="ob")
                        nc.vector.tensor_add(ob[:], acc[:], bias_sb[:, osl])
                        w = nc.sync.dma_start(out=partial[tsl, osl], in_=ob[:])
                        partial_dmas[qt].append(w)

            qk_proj(xq_st, wq_sb, qt_sb)
            for qt in range(NQT):
                attention(qt)
                # chunked reduce-scatter over this batch's 4 cores
                cc = nc.gpsimd.collective_compute(
                    "ReduceScatter",
                    mybir.AluOpType.add,
                    replica_groups=GROUPS,
                    ins=[partial[512 * qt:512 * (qt + 1), :]],
                    outs=[rsout[qt]],
                )
                for w in partial_dmas[qt]:
                    tile.add_dep_helper(cc.ins, w.ins, reason="RS after partial")
                # rsout chunk (bf16) -> sbuf -> fp32 -> out
                rs_sb = outsb.tile([128, D], bf16, tag="rs_sb")
                ld = nc.sync.dma_start(out=rs_sb[:], in_=rsout[qt])
                tile.add_dep_helper(ld.ins, cc.ins, reason="load after RS")
                rs32 = outsb.tile([128, D], f32, tag="rs32")
                nc.gpsimd.tensor_copy(rs32[:], rs_sb[:])
                nc.sync.dma_start(out=out[qt], in_=rs32[:])

    return nc


_CACHE = {}


def _get_program():
    if "nc" not in _CACHE:
        _CACHE["nc"] = build_program()
    return _CACHE["nc"]


def _bf16(x):
    import ml_dtypes
    return np.ascontiguousarray(np.asarray(x, dtype=ml_dtypes.bfloat16))


def make_in_maps(query, key, value, Wq, Wk, Wv, Wo, bo):
    """Host-side sharding: per-core input dicts."""
    query = np.asarray(query, dtype=np.float32)
    key = np.asarray(key, dtype=np.float32)
    value = np.asarray(value, dtype=np.float32)
    Wq = np.asarray(Wq, dtype=np.float32)
    Wk = np.asarray(Wk, dtype=np.float32)
    Wv = np.asarray(Wv, dtype=np.float32)
    Wo = np.asarray(Wo, dtype=np.float32)
    bo = np.asarray(bo, dtype=np.float32)

    xT = [_bf16(x.T) for x in
          (query[0], key[0], value[0], query[1], key[1], value[1])]
    bo4 = (bo * 0.25).astype(np.float32)
    wq_c, wk_c, wv_c, wo_c = [], [], [], []
    for hg in range(CPB):
        hsl = slice(HGD * hg, HGD * (hg + 1))
        wq_c.append(_bf16(Wq[hsl, :].T))
        wk_c.append(_bf16(Wk[hsl, :].T))
        wv_c.append(_bf16(Wv[hsl, :].T))
        wo_c.append(_bf16(Wo[:, hsl].T))
    in_maps = []
    for c in range(NCORES):
        b, hg = divmod(c, CPB)
        in_maps.append({
            "xqT": xT[3 * b + 0],
            "xkT": xT[3 * b + 1],
            "xvT": xT[3 * b + 2],
            "wqT": wq_c[hg],
            "wkT": wk_c[hg],
            "wvT": wv_c[hg],
            "woT": wo_c[hg],
            "bo4": bo4,
        })
    return in_maps


def assemble(results):
    """Each core owns strips 512*qt + 128*i (i = rank in its group)."""
    out = np.empty((B, NT, D), dtype=np.float32)
    for c in range(NCORES):
        b, i = divmod(c, CPB)
        o = results[c]["out"]
        for qt in range(NQT):
            s = 512 * qt + 128 * i
            out[b, s:s + 128, :] = o[qt]
    return out


def run(query, key, value, Wq, Wk, Wv, Wo, bo, trace=False):
    from concourse.bass_utils import run_bass_kernel_spmd
    nc = _get_program()
    in_maps = make_in_maps(query, key, value, Wq, Wk, Wv, Wo, bo)
    res = run_bass_kernel_spmd(nc, in_maps, core_ids=list(range(NCORES)),
                               trace=trace)
    return assemble(res.results), res


def kernel(query, key, value, qpos=None, kpos=None, Wq=None, Wk=None,
           Wv=None, Wo=None, bo=None):
    out, _ = run(query, key, value, Wq, Wk, Wv, Wo, bo)
    return out


# revision 8
# speedup vs baseline: 1.6912x; 1.0075x over previous
"""Trainium2 Bass kernel for nn_CrossAttention (B=2, N=2048, D=1024, H=16).

Sharding (8 cores): core c -> (batch b = c//4, head-group hg = c%4).
Each head-group is 4 heads = 256 of the 1024 projection dims.

v2 design (bf16 matmuls, pipelined attention, chunked overlapped RS):
  - All matmul inputs bf16 (host converts); PSUM accumulation fp32.
  - X^T staged fully resident in SBUF; all input DMAs issued upfront.
  - Scores for the 2 heads of a half-group go into ONE [128,1024] psum
    tile (2 banks) so exp is a single [128,1024] Activation instruction.
  - Inner kt loop software-pipelined: scores(kt+1) issued before PV(kt)
    so the PE never waits on the Activation engine.
  - Softmax denominators ride along as a ones-column in V (row 64 of the
    PV accumulator); normalization uses reciprocal_approx_fast (DVE) +
    DMA partition-broadcast; copies/bias-adds go to GpSimd (Pool).
  - Output projection per 512-token q-tile, ReduceScatter (bf16, add)
    per q-tile chunk overlapped with the next tile's attention. Core i
    of each 4-core group owns token strip 512*qt + 128*i.
"""

import numpy as np

B = 2
NT = 2048
D = 1024
HEADS = 16
DH = 64
NCORES = 8
CPB = 4  # cores per batch
HG = 4   # heads per core
HGD = HG * DH  # 256 cols per core
GROUPS = [[0, 1, 2, 3], [4, 5, 6, 7]]
SCALE = DH ** -0.5
NQT = 4     # q tiles of 512
NKT = 16    # kv tiles of 128
VW = DH + 1  # v columns per head incl. ones column

_patched = False


def _patch_tile_drain():
    """This container's walrus rejects >1 sync-wait on a Drain
    (CoreV3GenImpl setupSyncWait<CTRL_NO_STRUCT>: "Too many sync wait
    commands").  Split the final TileContext drain's waits across a chain
    of single-wait drains; semaphores are monotonic so sequential waits
    are equivalent to one multi-wait."""
    global _patched
    if _patched:
        return
    import concourse.tile as tile
    import concourse.mybir as mybir
    from concourse.vector_clock import ScopedClock

    _uid = [0]

    def _split_multiwaits(nc):
        for f in nc.m.functions:
            for bb in f.blocks:
                il = bb.instructions
                i = 0
                while i < len(il):
                    inst = il[i]
                    si = inst.sync_info
                    if si is not None and len(si.on_wait) > 1:
                        waits = list(si.on_wait)
                        inst.sync_info = mybir.SyncInfo(
                            on_wait=[waits[-1]], on_update=list(si.on_update)
                        )
                        for w in waits[:-1]:
                            _uid[0] += 1
                            nop = mybir.InstEventSemaphore(
                                name=f"WSPLIT-{_uid[0]}",
                                engine=inst.engine,
                                ins=[],
                                outs=[],
                                sync_info=mybir.SyncInfo(
                                    on_wait=[w], on_update=[]),
                            )
                            il.insert(i, nop)
                            i += 1
                    i += 1

    def _drain_and_barrier(self, tick_clock, wait_clock):
        nc = self.nc
        drain_inst = nc.sync.drain()
        wait_clock.add_sem_waits(
            drain_inst.ins, ScopedClock({None: tick_clock.global_clock})
        )
        si = drain_inst.ins.sync_info
        if si is not None and len(si.on_wait) > 1:
            waits = list(si.on_wait)
            drain_inst.ins.sync_info = mybir.SyncInfo(
                on_wait=[waits[0]], on_update=list(si.on_update)
            )
            for w in waits[1:]:
                extra = nc.sync.drain()
                extra.ins.sync_info = mybir.SyncInfo(on_wait=[w], on_update=[])

        _split_multiwaits(nc)
        nc.all_engine_barrier()
        assert self.sems is not None
        popped = nc._tile_sem_poison_stack.pop()
        assert popped is self._sem_poison
        nc.clear_and_free_semaphores(list(self.sems.allocated().values()))
        nc.all_engine_barrier()

    tile.TileContext._drain_and_barrier = _drain_and_barrier
    _patched = True


def build_program():
    _patch_tile_drain()
    import concourse.bass as bass
    import concourse.tile as tile
    import concourse.mybir as mybir

    f32 = mybir.dt.float32
    bf16 = mybir.dt.bfloat16
    EXP = mybir.ActivationFunctionType.Exp

    nc = bass.Bass("TRN2", target_bir_lowering=False, debug=False,
                   num_devices=NCORES)

    xqT = nc.dram_tensor("xqT", [D, NT], bf16, kind="ExternalInput")
    xkT = nc.dram_tensor("xkT", [D, NT], bf16, kind="ExternalInput")
    xvT = nc.dram_tensor("xvT", [D, NT], bf16, kind="ExternalInput")
    wqT = nc.dram_tensor("wqT", [D, HGD], bf16, kind="ExternalInput")
    wkT = nc.dram_tensor("wkT", [D, HGD], bf16, kind="ExternalInput")
    wvT = nc.dram_tensor("wvT", [D, HGD], bf16, kind="ExternalInput")
    woT = nc.dram_tensor("woT", [HGD, D], bf16, kind="ExternalInput")
    bo4 = nc.dram_tensor("bo4", [D], f32, kind="ExternalInput")
    out = nc.dram_tensor("out", [NQT, 128, D], f32, kind="ExternalOutput")

    partial = nc.dram_tensor("partial", [NT, D], bf16)
    rsout = nc.dram_tensor("rsout", [NQT, 128, D], bf16)
    rbounce = nc.dram_tensor("rbounce", [16, 512], f32)

    with tile.TileContext(nc) as tc:
        from contextlib import ExitStack
        with ExitStack() as ctx:
            const = ctx.enter_context(tc.tile_pool(name="const", bufs=1))
            persist = ctx.enter_context(tc.tile_pool(name="persist", bufs=1))
            pt_pool = ctx.enter_context(tc.tile_pool(name="pt", bufs=3))
            normp = ctx.enter_context(tc.tile_pool(name="normp", bufs=2))
            outsb = ctx.enter_context(tc.tile_pool(name="outsb", bufs=2))
            # PSUM: 8 banks of [128, 2KB]. st tiles are [128,1024]f32 =
            # 2 banks each (scores for 2 heads side by side; also reused
            # as projection accumulators). xa = attention x^T accums,
            # op = out-projection accumulators.
            st_ps = ctx.enter_context(
                tc.tile_pool(name="st_ps", bufs=2, space="PSUM"))
            xa_ps = ctx.enter_context(
                tc.tile_pool(name="xa_ps", bufs=2, space="PSUM"))
            op_ps = ctx.enter_context(
                tc.tile_pool(name="op_ps", bufs=2, space="PSUM"))

            # --- constants + all input DMAs upfront ------------------------
            wq_sb = const.tile([128, 8, HGD], bf16)  # [k-part, k-tile, col]
            wk_sb = const.tile([128, 8, HGD], bf16)
            wv_sb = const.tile([128, 8, HGD], bf16)
            wo_sb = const.tile([128, 2, D], bf16)    # [d-part, hg k-tile, od]
            bias_sb = const.tile([128, D], f32)      # bo/4 bcast over parts
            nc.sync.dma_start(out=wk_sb[:], in_=wkT[:].rearrange(
                "(t p) c -> p t c", p=128))
            nc.sync.dma_start(out=wv_sb[:], in_=wvT[:].rearrange(
                "(t p) c -> p t c", p=128))
            nc.sync.dma_start(out=wq_sb[:], in_=wqT[:].rearrange(
                "(t p) c -> p t c", p=128))
            nc.sync.dma_start(out=wo_sb[:], in_=woT[:].rearrange(
                "(t p) c -> p t c", p=128))
            nc.sync.dma_start(out=bias_sb[:],
                              in_=bo4[:].partition_broadcast(128))

            # resident X^T staging: [part, k-tile, token]
            xk_st = persist.tile([128, 8, NT], bf16)
            xv_st = persist.tile([128, 8, NT], bf16)
            xq_st = persist.tile([128, 8, NT], bf16)
            for n in range(4):
                sl = slice(512 * n, 512 * (n + 1))
                nc.sync.dma_start(out=xk_st[:, :, sl], in_=xkT[:, sl]
                                  .rearrange("(t p) c -> p t c", p=128))
            for n in range(4):
                sl = slice(512 * n, 512 * (n + 1))
                nc.sync.dma_start(out=xv_st[:, :, sl], in_=xvT[:, sl]
                                  .rearrange("(t p) c -> p t c", p=128))
            for n in range(4):
                sl = slice(512 * n, 512 * (n + 1))
                nc.sync.dma_start(out=xq_st[:, :, sl], in_=xqT[:, sl]
                                  .rearrange("(t p) c -> p t c", p=128))

            # --- persistent activations -----------------------------------
            qt_sb = persist.tile([128, 2, NT], bf16)  # [qcol%128, qcol//128, tok]
            kt_sb = persist.tile([128, 2, NT], bf16)
            v_sb = persist.tile([128, NKT, HG * VW], bf16)  # + ones cols
            xt_sb = persist.tile([128, 2, NT], bf16)  # normalized x^T

            nc.vector.memset(v_sb[:], 1.0)

            # --- K projection: KT[qcol, tok] = sum_k W^T[k,qcol] X^T[k,tok]
            def qk_proj(xst, wsb, dst):
                for n in range(NQT):
                    acc = st_ps.tile([128, 1024], f32, tag="st", name="qkacc")
                    for k in range(8):
                        for m in range(2):
                            nc.tensor.matmul(
                                acc[:, 512 * m:512 * (m + 1)],
                                wsb[:, k, 128 * m:128 * (m + 1)],
                                xst[:, k, 512 * n:512 * (n + 1)],
                                start=(k == 0), stop=(k == 7))
                    for m in range(2):
                        nc.vector.tensor_copy(
                            dst[:, m, 512 * n:512 * (n + 1)],
                            acc[:, 512 * m:512 * (m + 1)])

            qk_proj(xk_st, wk_sb, kt_sb)

            # --- V projection: V[tok, vcol] = sum_k X^T[k,tok] W^T[k,vcol]
            for mg in range(4):
                acc = st_ps.tile([128, 1024], f32, tag="st", name="vacc")
                # m outer / k inner: regions m=0,1 (and 2,3) share a psum
                # bank; interleaving two open accumulation groups in one
                # bank corrupts results, so finish each region first.
                for m in range(4):
                    for k in range(8):
                        nc.tensor.matmul(
                            acc[:, 256 * m:256 * m + HGD],
                            xv_st[:, k, 512 * mg + 128 * m:
                                  512 * mg + 128 * (m + 1)],
                            wv_sb[:, k, :],
                            start=(k == 0), stop=(k == 7))
                for m in range(4):
                    mt = 4 * mg + m
                    # strided copy: drop into [128, HG, DH] slots of v_sb,
                    # leaving the ones columns (index DH) intact
                    nc.vector.tensor_copy(
                        v_sb[:, mt, :].rearrange("p (h w) -> p h w", w=VW)
                        [:, :, 0:DH],
                        acc[:, 256 * m:256 * (m + 1)].rearrange(
                            "p (h w) -> p h w", w=DH))

            # --- attention (per q-tile, per head-pair) ---------------------
            partial_dmas = [[] for _ in range(NQT)]
            rs_ccs = []

            def att_hp(qt, hp):
                qsl = slice(512 * qt, 512 * (qt + 1))
                xa = [xa_ps.tile([DH + 1, 512], f32, tag="xa", name="xa")
                      for _ in range(2)]

                def scores(kt):
                    st = st_ps.tile([128, 1024], f32, tag="st", name="st")
                    for j in range(2):
                        p0 = 64 * j
                        nc.tensor.matmul(
                            st[:, 512 * j:512 * (j + 1)],
                            kt_sb[p0:p0 + 64, hp,
                                  128 * kt:128 * (kt + 1)],
                            qt_sb[p0:p0 + 64, hp, qsl],
                            tile_position=(p0, 0))
                    return st

                def exp(st):
                    pt = pt_pool.tile([128, 1024], bf16)
                    nc.scalar.activation(pt[:], st[:], EXP, scale=SCALE)
                    return pt

                def pv(kt, pt):
                    for j in range(2):
                        h = 2 * hp + j
                        nc.tensor.matmul(
                            xa[j][:],
                            v_sb[:, kt, VW * h:VW * (h + 1)],
                            pt[:, 512 * j:512 * (j + 1)],
                            start=(kt == 0), stop=(kt == NKT - 1))

                pt_prev = exp(scores(0))
                for kt in range(1, NKT):
                    st = scores(kt)
                    pv(kt - 1, pt_prev)
                    pt_prev = exp(st)
                pv(NKT - 1, pt_prev)

                # normalize: x[d, tok] /= denom[tok] (denoms in row DH).
                # Reciprocal is slow per free-dim element on DVE, so reshape
                # the 512 denominators to [128, 4] via DMA first.
                for j in range(2):
                    dnr = normp.tile([DH + 1, 512], f32, tag=f"dnr{j}")
                    nc.vector.tensor_copy(dnr[DH:DH + 1, :],
                                          xa[j][DH:DH + 1, :])
                    rs4 = normp.tile([128, 4], f32, tag=f"rs4_{j}")
                    nc.sync.dma_start(out=rs4[:], in_=dnr[DH:DH + 1, :])
                    rc4 = normp.tile([128, 4], f32, tag=f"rc4_{j}")
                    nc.vector.reciprocal(rc4[:], rs4[:])
                    ridx = 4 * (qt % 2) + 2 * hp + j
                    rb = rbounce[ridx:ridx + 1, :]
                    nc.sync.dma_start(out=rb, in_=rc4[:])
                    bc = normp.tile([DH, 512], f32, tag=f"bc{j}")
                    nc.sync.dma_start(out=bc[:],
                                      in_=rb.partition_broadcast(DH))
                    if j == 0:
                        nc.vector.tensor_mul(
                            xt_sb[0:DH, hp, qsl], xa[j][0:DH, :], bc[:])
                    else:
                        tm = normp.tile([DH, 512], bf16, tag="tm")
                        nc.vector.tensor_mul(tm[:], xa[j][0:DH, :], bc[:])
                        nc.sync.dma_start(out=xt_sb[DH:128, hp, qsl],
                                          in_=tm[:])

            def outproj_rs(qt):
                # out-proj: partial[t, o] = sum_d x^T[d, t] Wo^T[d, o] + bo/4
                for n in range(2):
                    osl = slice(512 * n, 512 * (n + 1))
                    for m in range(4):
                        tsl = slice(512 * qt + 128 * m,
                                    512 * qt + 128 * (m + 1))
                        acc = op_ps.tile([128, 512], f32, tag="op")
                        for k in range(2):
                            nc.tensor.matmul(
                                acc[:],
                                xt_sb[:, k, tsl],
                                wo_sb[:, k, osl],
                                start=(k == 0), stop=(k == 1))
                        ob = outsb.tile([128, 512], bf16, tag="ob")
                        nc.vector.tensor_add(ob[:], acc[:], bias_sb[:, osl])
                        w = nc.sync.dma_start(out=partial[tsl, osl], in_=ob[:])
                        partial_dmas[qt].append(w)
                # chunked reduce-scatter over this batch's 4 cores
                cc = nc.gpsimd.collective_compute(
                    "ReduceScatter",
                    mybir.AluOpType.add,
                    replica_groups=GROUPS,
                    ins=[partial[512 * qt:512 * (qt + 1), :]],
                    outs=[rsout[qt]],
                )
                for w in partial_dmas[qt]:
                    tile.add_dep_helper(cc.ins, w.ins, reason="RS after partial")
                rs_ccs.append(cc)

            qk_proj(xq_st, wq_sb, qt_sb)
            # schedule: out-proj/RS of tile qt-1 is sandwiched between the
            # two head-pair blocks of tile qt, so the PE never waits on the
            # normalize chain, and each RS overlaps the next tile's compute.
            att_hp(0, 0)
            att_hp(0, 1)
            for qt in range(1, NQT):
                att_hp(qt, 0)
                outproj_rs(qt - 1)
                att_hp(qt, 1)
            outproj_rs(NQT - 1)

            # tails at the very end: the in-order SP queue would otherwise
            # head-block on each RS completion, stalling every later DMA.
            for qt in range(NQT):
                rs_sb = outsb.tile([128, D], bf16, tag="rs_sb")
                ld = nc.sync.dma_start(out=rs_sb[:], in_=rsout[qt])
                tile.add_dep_helper(ld.ins, rs_ccs[qt].ins,
                                    reason="load after RS")
                rs32 = outsb.tile([128, D], f32, tag="rs32")
                nc.vector.tensor_copy(rs32[:], rs_sb[:])
                nc.sync.dma_start(out=out[qt], in_=rs32[:])

    return nc


_CACHE = {}


def _get_program():
    if "nc" not in _CACHE:
        _CACHE["nc"] = build_program()
    return _CACHE["nc"]


def _bf16(x):
    import ml_dtypes
    return np.ascontiguousarray(np.asarray(x, dtype=ml_dtypes.bfloat16))


def make_in_maps(query, key, value, Wq, Wk, Wv, Wo, bo):
    """Host-side sharding: per-core input dicts."""
    query = np.asarray(query, dtype=np.float32)
    key = np.asarray(key, dtype=np.float32)
    value = np.asarray(value, dtype=np.float32)
    Wq = np.asarray(Wq, dtype=np.float32)
    Wk = np.asarray(Wk, dtype=np.float32)
    Wv = np.asarray(Wv, dtype=np.float32)
    Wo = np.asarray(Wo, dtype=np.float32)
    bo = np.asarray(bo, dtype=np.float32)

    xT = [_bf16(x.T) for x in
          (query[0], key[0], value[0], query[1], key[1], value[1])]
    bo4 = (bo * 0.25).astype(np.float32)
    wq_c, wk_c, wv_c, wo_c = [], [], [], []
    for hg in range(CPB):
        hsl = slice(HGD * hg, HGD * (hg + 1))
        wq_c.append(_bf16(Wq[hsl, :].T))
        wk_c.append(_bf16(Wk[hsl, :].T))
        wv_c.append(_bf16(Wv[hsl, :].T))
        wo_c.append(_bf16(Wo[:, hsl].T))
    in_maps = []
    for c in range(NCORES):
        b, hg = divmod(c, CPB)
        in_maps.append({
            "xqT": xT[3 * b + 0],
            "xkT": xT[3 * b + 1],
            "xvT": xT[3 * b + 2],
            "wqT": wq_c[hg],
            "wkT": wk_c[hg],
            "wvT": wv_c[hg],
            "woT": wo_c[hg],
            "bo4": bo4,
        })
    return in_maps


def assemble(results):
    """Each core owns strips 512*qt + 128*i (i = rank in its group)."""
    out = np.empty((B, NT, D), dtype=np.float32)
    for c in range(NCORES):
        b, i = divmod(c, CPB)
        o = results[c]["out"]
        for qt in range(NQT):
            s = 512 * qt + 128 * i
            out[b, s:s + 128, :] = o[qt]
    return out


def run(query, key, value, Wq, Wk, Wv, Wo, bo, trace=False):
    from concourse.bass_utils import run_bass_kernel_spmd
    nc = _get_program()
    in_maps = make_in_maps(query, key, value, Wq, Wk, Wv, Wo, bo)
    res = run_bass_kernel_spmd(nc, in_maps, core_ids=list(range(NCORES)),
                               trace=trace)
    return assemble(res.results), res


def kernel(query, key, value, qpos=None, kpos=None, Wq=None, Wk=None,
           Wv=None, Wo=None, bo=None):
    out, _ = run(query, key, value, Wq, Wk, Wv, Wo, bo)
    return out


# revision 9
# speedup vs baseline: 1.7328x; 1.0246x over previous
"""Trainium2 Bass kernel for nn_CrossAttention (B=2, N=2048, D=1024, H=16).

Sharding (8 cores): core c -> (batch b = c//4, head-group hg = c%4).
Each head-group is 4 heads = 256 of the 1024 projection dims.

v2 design (bf16 matmuls, pipelined attention, chunked overlapped RS):
  - All matmul inputs bf16 (host converts); PSUM accumulation fp32.
  - X^T staged fully resident in SBUF; all input DMAs issued upfront.
  - Scores for the 2 heads of a half-group go into ONE [128,1024] psum
    tile (2 banks) so exp is a single [128,1024] Activation instruction.
  - Inner kt loop software-pipelined: scores(kt+1) issued before PV(kt)
    so the PE never waits on the Activation engine.
  - Softmax denominators ride along as a ones-column in V (row 64 of the
    PV accumulator); normalization uses reciprocal_approx_fast (DVE) +
    DMA partition-broadcast; copies/bias-adds go to GpSimd (Pool).
  - Output projection per 512-token q-tile, ReduceScatter (bf16, add)
    per q-tile chunk overlapped with the next tile's attention. Core i
    of each 4-core group owns token strip 512*qt + 128*i.
"""

import numpy as np

B = 2
NT = 2048
D = 1024
HEADS = 16
DH = 64
NCORES = 8
CPB = 4  # cores per batch
HG = 4   # heads per core
HGD = HG * DH  # 256 cols per core
GROUPS = [[0, 1, 2, 3], [4, 5, 6, 7]]
SCALE = DH ** -0.5
NQT = 4     # q tiles of 512
NKT = 16    # kv tiles of 128
VW = DH + 1  # v columns per head incl. ones column

_patched = False


def _patch_tile_drain():
    """This container's walrus rejects >1 sync-wait on a Drain
    (CoreV3GenImpl setupSyncWait<CTRL_NO_STRUCT>: "Too many sync wait
    commands").  Split the final TileContext drain's waits across a chain
    of single-wait drains; semaphores are monotonic so sequential waits
    are equivalent to one multi-wait."""
    global _patched
    if _patched:
        return
    import concourse.tile as tile
    import concourse.mybir as mybir
    from concourse.vector_clock import ScopedClock

    _uid = [0]

    def _split_multiwaits(nc):
        for f in nc.m.functions:
            for bb in f.blocks:
                il = bb.instructions
                i = 0
                while i < len(il):
                    inst = il[i]
                    si = inst.sync_info
                    if si is not None and len(si.on_wait) > 1:
                        waits = list(si.on_wait)
                        inst.sync_info = mybir.SyncInfo(
                            on_wait=[waits[-1]], on_update=list(si.on_update)
                        )
                        for w in waits[:-1]:
                            _uid[0] += 1
                            nop = mybir.InstEventSemaphore(
                                name=f"WSPLIT-{_uid[0]}",
                                engine=inst.engine,
                                ins=[],
                                outs=[],
                                sync_info=mybir.SyncInfo(
                                    on_wait=[w], on_update=[]),
                            )
                            il.insert(i, nop)
                            i += 1
                    i += 1

    def _drain_and_barrier(self, tick_clock, wait_clock):
        nc = self.nc
        drain_inst = nc.sync.drain()
        wait_clock.add_sem_waits(
            drain_inst.ins, ScopedClock({None: tick_clock.global_clock})
        )
        si = drain_inst.ins.sync_info
        if si is not None and len(si.on_wait) > 1:
            waits = list(si.on_wait)
            drain_inst.ins.sync_info = mybir.SyncInfo(
                on_wait=[waits[0]], on_update=list(si.on_update)
            )
            for w in waits[1:]:
                extra = nc.sync.drain()
                extra.ins.sync_info = mybir.SyncInfo(on_wait=[w], on_update=[])

        _split_multiwaits(nc)
        nc.all_engine_barrier()
        assert self.sems is not None
        popped = nc._tile_sem_poison_stack.pop()
        assert popped is self._sem_poison
        nc.clear_and_free_semaphores(list(self.sems.allocated().values()))
        nc.all_engine_barrier()

    tile.TileContext._drain_and_barrier = _drain_and_barrier
    _patched = True


def build_program():
    _patch_tile_drain()
    import concourse.bass as bass
    import concourse.tile as tile
    import concourse.mybir as mybir

    f32 = mybir.dt.float32
    bf16 = mybir.dt.bfloat16
    EXP = mybir.ActivationFunctionType.Exp

    nc = bass.Bass("TRN2", target_bir_lowering=False, debug=False,
                   num_devices=NCORES)

    xqT = nc.dram_tensor("xqT", [D, NT], bf16, kind="ExternalInput")
    xkT = nc.dram_tensor("xkT", [D, NT], bf16, kind="ExternalInput")
    xvT = nc.dram_tensor("xvT", [D, NT], bf16, kind="ExternalInput")
    wqT = nc.dram_tensor("wqT", [D, HGD], bf16, kind="ExternalInput")
    wkT = nc.dram_tensor("wkT", [D, HGD], bf16, kind="ExternalInput")
    wvT = nc.dram_tensor("wvT", [D, HGD], bf16, kind="ExternalInput")
    woT = nc.dram_tensor("woT", [HGD, D], bf16, kind="ExternalInput")
    out = nc.dram_tensor("out", [NQT, 128, D], f32, kind="ExternalOutput")

    partial = nc.dram_tensor("partial", [NT, D], bf16)
    rsout = nc.dram_tensor("rsout", [NQT, 128, D], bf16)
    rbounce = nc.dram_tensor("rbounce", [16, 512], f32)

    with tile.TileContext(nc) as tc:
        from contextlib import ExitStack
        with ExitStack() as ctx:
            const = ctx.enter_context(tc.tile_pool(name="const", bufs=1))
            persist = ctx.enter_context(tc.tile_pool(name="persist", bufs=1))
            pt_pool = ctx.enter_context(tc.tile_pool(name="pt", bufs=3))
            normp = ctx.enter_context(tc.tile_pool(name="normp", bufs=2))
            outsb = ctx.enter_context(tc.tile_pool(name="outsb", bufs=2))
            # PSUM: 8 banks of [128, 2KB]. st tiles are [128,1024]f32 =
            # 2 banks each (scores for 2 heads side by side; also reused
            # as projection accumulators). xa = attention x^T accums,
            # op = out-projection accumulators.
            st_ps = ctx.enter_context(
                tc.tile_pool(name="st_ps", bufs=2, space="PSUM"))
            xa_ps = ctx.enter_context(
                tc.tile_pool(name="xa_ps", bufs=2, space="PSUM"))
            op_ps = ctx.enter_context(
                tc.tile_pool(name="op_ps", bufs=2, space="PSUM"))

            # --- constants + all input DMAs upfront ------------------------
            wq_sb = const.tile([128, 8, HGD], bf16)  # [k-part, k-tile, col]
            wk_sb = const.tile([128, 8, HGD], bf16)
            wv_sb = const.tile([128, 8, HGD], bf16)
            wo_sb = const.tile([128, 2, D], bf16)    # [d-part, hg k-tile, od]
            xk_st = persist.tile([128, 8, NT], bf16)
            xv_st = persist.tile([128, 8, NT], bf16)
            xq_st = persist.tile([128, 8, NT], bf16)
            # DMA issue order = consumption order so compute starts early
            nc.sync.dma_start(out=wk_sb[:], in_=wkT[:].rearrange(
                "(t p) c -> p t c", p=128))
            for n in range(4):
                sl = slice(512 * n, 512 * (n + 1))
                nc.sync.dma_start(out=xk_st[:, :, sl], in_=xkT[:, sl]
                                  .rearrange("(t p) c -> p t c", p=128))
            nc.sync.dma_start(out=wv_sb[:], in_=wvT[:].rearrange(
                "(t p) c -> p t c", p=128))
            for n in range(4):
                sl = slice(512 * n, 512 * (n + 1))
                nc.sync.dma_start(out=xv_st[:, :, sl], in_=xvT[:, sl]
                                  .rearrange("(t p) c -> p t c", p=128))
            nc.sync.dma_start(out=wq_sb[:], in_=wqT[:].rearrange(
                "(t p) c -> p t c", p=128))
            for n in range(4):
                sl = slice(512 * n, 512 * (n + 1))
                nc.sync.dma_start(out=xq_st[:, :, sl], in_=xqT[:, sl]
                                  .rearrange("(t p) c -> p t c", p=128))
            nc.sync.dma_start(out=wo_sb[:], in_=woT[:].rearrange(
                "(t p) c -> p t c", p=128))

            # --- persistent activations -----------------------------------
            qt_sb = persist.tile([128, 2, NT], bf16)  # [qcol%128, qcol//128, tok]
            kt_sb = persist.tile([128, 2, NT], bf16)
            v_sb = persist.tile([128, NKT, HG * VW], bf16)  # + ones cols
            xt_sb = persist.tile([128, 2, NT], bf16)  # normalized x^T

            nc.vector.memset(v_sb[:], 1.0)

            # --- K projection: KT[qcol, tok] = sum_k W^T[k,qcol] X^T[k,tok]
            def qk_proj(xst, wsb, dst):
                for n in range(NQT):
                    acc = st_ps.tile([128, 1024], f32, tag="st", name="qkacc")
                    for k in range(8):
                        for m in range(2):
                            nc.tensor.matmul(
                                acc[:, 512 * m:512 * (m + 1)],
                                wsb[:, k, 128 * m:128 * (m + 1)],
                                xst[:, k, 512 * n:512 * (n + 1)],
                                start=(k == 0), stop=(k == 7))
                    for m in range(2):
                        nc.vector.tensor_copy(
                            dst[:, m, 512 * n:512 * (n + 1)],
                            acc[:, 512 * m:512 * (m + 1)])

            qk_proj(xk_st, wk_sb, kt_sb)

            # --- V projection: V[tok, vcol] = sum_k X^T[k,tok] W^T[k,vcol]
            for mg in range(4):
                acc = st_ps.tile([128, 1024], f32, tag="st", name="vacc")
                # m outer / k inner: regions m=0,1 (and 2,3) share a psum
                # bank; interleaving two open accumulation groups in one
                # bank corrupts results, so finish each region first.
                for m in range(4):
                    for k in range(8):
                        nc.tensor.matmul(
                            acc[:, 256 * m:256 * m + HGD],
                            xv_st[:, k, 512 * mg + 128 * m:
                                  512 * mg + 128 * (m + 1)],
                            wv_sb[:, k, :],
                            start=(k == 0), stop=(k == 7))
                for m in range(4):
                    mt = 4 * mg + m
                    # strided copy: drop into [128, HG, DH] slots of v_sb,
                    # leaving the ones columns (index DH) intact
                    nc.vector.tensor_copy(
                        v_sb[:, mt, :].rearrange("p (h w) -> p h w", w=VW)
                        [:, :, 0:DH],
                        acc[:, 256 * m:256 * (m + 1)].rearrange(
                            "p (h w) -> p h w", w=DH))

            # --- attention (per q-tile, per head-pair) ---------------------
            partial_dmas = [[] for _ in range(NQT)]
            rs_ccs = []

            def att_hp(qt, hp):
                qsl = slice(512 * qt, 512 * (qt + 1))
                xa = [xa_ps.tile([DH + 1, 512], f32, tag="xa", name="xa")
                      for _ in range(2)]

                def scores(kt):
                    st = st_ps.tile([128, 1024], f32, tag="st", name="st")
                    for j in range(2):
                        p0 = 64 * j
                        nc.tensor.matmul(
                            st[:, 512 * j:512 * (j + 1)],
                            kt_sb[p0:p0 + 64, hp,
                                  128 * kt:128 * (kt + 1)],
                            qt_sb[p0:p0 + 64, hp, qsl],
                            tile_position=(p0, 0))
                    return st

                def exp(st):
                    pt = pt_pool.tile([128, 1024], bf16)
                    nc.scalar.activation(pt[:], st[:], EXP, scale=SCALE)
                    return pt

                def pv(kt, pt):
                    for j in range(2):
                        h = 2 * hp + j
                        nc.tensor.matmul(
                            xa[j][:],
                            v_sb[:, kt, VW * h:VW * (h + 1)],
                            pt[:, 512 * j:512 * (j + 1)],
                            start=(kt == 0), stop=(kt == NKT - 1))

                pt_prev = exp(scores(0))
                for kt in range(1, NKT):
                    st = scores(kt)
                    pv(kt - 1, pt_prev)
                    pt_prev = exp(st)
                pv(NKT - 1, pt_prev)

                # normalize: x[d, tok] /= denom[tok] (denoms in row DH).
                # Reciprocal is slow per free-dim element on DVE, so reshape
                # the 512 denominators to [128, 4] via DMA first.
                for j in range(2):
                    dnr = normp.tile([DH + 1, 512], f32, tag=f"dnr{j}")
                    nc.vector.tensor_copy(dnr[DH:DH + 1, :],
                                          xa[j][DH:DH + 1, :])
                    rs4 = normp.tile([128, 4], f32, tag=f"rs4_{j}")
                    nc.sync.dma_start(out=rs4[:], in_=dnr[DH:DH + 1, :])
                    rc4 = normp.tile([128, 4], f32, tag=f"rc4_{j}")
                    nc.vector.reciprocal(rc4[:], rs4[:])
                    ridx = 4 * (qt % 2) + 2 * hp + j
                    rb = rbounce[ridx:ridx + 1, :]
                    nc.sync.dma_start(out=rb, in_=rc4[:])
                    bc = normp.tile([DH, 512], f32, tag=f"bc{j}")
                    nc.sync.dma_start(out=bc[:],
                                      in_=rb.partition_broadcast(DH))
                    if j == 0:
                        nc.vector.tensor_mul(
                            xt_sb[0:DH, hp, qsl], xa[j][0:DH, :], bc[:])
                    else:
                        tm = normp.tile([DH, 512], bf16, tag="tm")
                        nc.vector.tensor_mul(tm[:], xa[j][0:DH, :], bc[:])
                        nc.sync.dma_start(out=xt_sb[DH:128, hp, qsl],
                                          in_=tm[:])

            def outproj_rs(qt):
                # out-proj: partial[t, o] = sum_d x^T[d, t] Wo^T[d, o] + bo/4
                for n in range(2):
                    osl = slice(512 * n, 512 * (n + 1))
                    for m in range(4):
                        tsl = slice(512 * qt + 128 * m,
                                    512 * qt + 128 * (m + 1))
                        acc = op_ps.tile([128, 512], f32, tag="op")
                        for k in range(2):
                            nc.tensor.matmul(
                                acc[:],
                                xt_sb[:, k, tsl],
                                wo_sb[:, k, osl],
                                start=(k == 0), stop=(k == 1))
                        ob = outsb.tile([128, 512], bf16, tag="ob")
                        nc.scalar.copy(ob[:], acc[:])
                        w = nc.sync.dma_start(out=partial[tsl, osl], in_=ob[:])
                        partial_dmas[qt].append(w)
                # chunked reduce-scatter over this batch's 4 cores
                cc = nc.gpsimd.collective_compute(
                    "ReduceScatter",
                    mybir.AluOpType.add,
                    replica_groups=GROUPS,
                    ins=[partial[512 * qt:512 * (qt + 1), :]],
                    outs=[rsout[qt]],
                )
                for w in partial_dmas[qt]:
                    tile.add_dep_helper(cc.ins, w.ins, reason="RS after partial")
                rs_ccs.append(cc)

            qk_proj(xq_st, wq_sb, qt_sb)
            # schedule: out-proj/RS of tile qt-1 is sandwiched between the
            # two head-pair blocks of tile qt, so the PE never waits on the
            # normalize chain, and each RS overlaps the next tile's compute.
            att_hp(0, 0)
            att_hp(0, 1)
            for qt in range(1, NQT):
                att_hp(qt, 0)
                outproj_rs(qt - 1)
                att_hp(qt, 1)
            outproj_rs(NQT - 1)

            # tails at the very end: the in-order SP queue would otherwise
            # head-block on each RS completion, stalling every later DMA.
            for qt in range(NQT):
                rs_sb = outsb.tile([128, D], bf16, tag="rs_sb")
                ld = nc.sync.dma_start(out=rs_sb[:], in_=rsout[qt])
                tile.add_dep_helper(ld.ins, rs_ccs[qt].ins,
                                    reason="load after RS")
                rs32 = outsb.tile([128, D], f32, tag="rs32")
                nc.vector.tensor_copy(rs32[:], rs_sb[:])
                nc.sync.dma_start(out=out[qt], in_=rs32[:])

    return nc


_CACHE = {}


def _get_program():
    if "nc" not in _CACHE:
        _CACHE["nc"] = build_program()
    return _CACHE["nc"]


def _bf16(x):
    import ml_dtypes
    return np.ascontiguousarray(np.asarray(x, dtype=ml_dtypes.bfloat16))


def make_in_maps(query, key, value, Wq, Wk, Wv, Wo, bo):
    """Host-side sharding: per-core input dicts."""
    query = np.asarray(query, dtype=np.float32)
    key = np.asarray(key, dtype=np.float32)
    value = np.asarray(value, dtype=np.float32)
    Wq = np.asarray(Wq, dtype=np.float32)
    Wk = np.asarray(Wk, dtype=np.float32)
    Wv = np.asarray(Wv, dtype=np.float32)
    Wo = np.asarray(Wo, dtype=np.float32)
    bo = np.asarray(bo, dtype=np.float32)

    xT = [_bf16(x.T) for x in
          (query[0], key[0], value[0], query[1], key[1], value[1])]
    wq_c, wk_c, wv_c, wo_c = [], [], [], []
    for hg in range(CPB):
        hsl = slice(HGD * hg, HGD * (hg + 1))
        wq_c.append(_bf16(Wq[hsl, :].T))
        wk_c.append(_bf16(Wk[hsl, :].T))
        wv_c.append(_bf16(Wv[hsl, :].T))
        wo_c.append(_bf16(Wo[:, hsl].T))
    in_maps = []
    for c in range(NCORES):
        b, hg = divmod(c, CPB)
        in_maps.append({
            "xqT": xT[3 * b + 0],
            "xkT": xT[3 * b + 1],
            "xvT": xT[3 * b + 2],
            "wqT": wq_c[hg],
            "wkT": wk_c[hg],
            "wvT": wv_c[hg],
            "woT": wo_c[hg],
        })
    return in_maps


def assemble(results):
    """Each core owns strips 512*qt + 128*i (i = rank in its group)."""
    out = np.empty((B, NT, D), dtype=np.float32)
    for c in range(NCORES):
        b, i = divmod(c, CPB)
        o = results[c]["out"]
        for qt in range(NQT):
            s = 512 * qt + 128 * i
            out[b, s:s + 128, :] = o[qt]
    return out


def run(query, key, value, Wq, Wk, Wv, Wo, bo, trace=False):
    from concourse.bass_utils import run_bass_kernel_spmd
    nc = _get_program()
    in_maps = make_in_maps(query, key, value, Wq, Wk, Wv, Wo, bo)
    res = run_bass_kernel_spmd(nc, in_maps, core_ids=list(range(NCORES)),
                               trace=trace)
    out = assemble(res.results)
    out += np.asarray(bo, dtype=np.float32)
    return out, res


def kernel(query, key, value, qpos=None, kpos=None, Wq=None, Wk=None,
           Wv=None, Wo=None, bo=None):
    out, _ = run(query, key, value, Wq, Wk, Wv, Wo, bo)
    return out


# revision 12
# speedup vs baseline: 1.9649x; 1.1339x over previous
"""Trainium2 Bass kernel for nn_CrossAttention (B=2, N=2048, D=1024, H=16).

Sharding (8 cores): core c -> (batch b = c//4, head-group hg = c%4).
Each head-group is 4 heads = 256 of the 1024 projection dims.

v5 design:
  - All matmul inputs bf16 (host converts); PSUM accumulation fp32.
  - X^T staged fully resident in SBUF; input DMAs upfront, split in two
    per 512-token chunk so the first projection starts early.
  - Scores for the 2 heads of a half-group go into ONE [128,1024] psum
    tile (2 banks) so exp is a single [128,1024] Activation instruction.
  - Inner kt loop software-pipelined: scores(kt+1) issued before PV(kt).
  - V layout per (kv-tile, head-pair): [v_h0 | ones | v_h1] (129 cols).
    j=0 PV lhsT = cols 0..64  -> xa0[0:65]   (x rows 0-63, denom row 64)
    j=1 PV lhsT = cols 1..128 -> xa1[0:128]  (denom row 63, x rows 64-127)
    so BOTH head outputs land partition-aligned for the DVE multiplies
    and no partition-shift DMA is needed.
  - Softmax normalize is engine-only (immune to DMA/collective traffic):
    copy xa->sbuf (frees psum fast), DVE 32x32 stream-transpose reshapes
    the 512 denominators to [32,16], reciprocal there (cheap), transpose
    back, then a K=1 PE matmul broadcasts the bf16 reciprocal row into a
    [128,512] psum tile (both heads at tile_position cols 0/64).  The
    broadcast matmuls + multiplies are deferred into the next block's
    kt loop so the PE never waits on the DVE chain.
  - Out-projection reuses the score psum pool; bias bo is added on host.
  - Per-q-tile bf16 ReduceScatter overlapped with the next tile's
    compute; the last tile is split into 2x256-token chunks to shorten
    the exposed tail.  Tail loads/conversions all sit at program end so
    the in-order SP DMA queue never head-blocks on a collective.
"""

import numpy as np

B = 2
NT = 2048
D = 1024
HEADS = 16
DH = 64
NCORES = 8
CPB = 4  # cores per batch
HG = 4   # heads per core
HGD = HG * DH  # 256 cols per core
GROUPS = [[0, 1, 2, 3], [4, 5, 6, 7]]
SCALE = DH ** -0.5
NQT = 4     # q tiles of 512
NKT = 16    # kv tiles of 128
VB = 2 * DH + 1  # v columns per head-pair block: [v_h0 | ones | v_h1]

_patched = False


def _patch_tile_drain():
    """This container's walrus rejects >1 sync-wait on a Drain
    (CoreV3GenImpl setupSyncWait<CTRL_NO_STRUCT>: "Too many sync wait
    commands").  Split the final TileContext drain's waits across a chain
    of single-wait drains; semaphores are monotonic so sequential waits
    are equivalent to one multi-wait."""
    global _patched
    if _patched:
        return
    import concourse.tile as tile
    import concourse.mybir as mybir
    from concourse.vector_clock import ScopedClock

    _uid = [0]

    def _split_multiwaits(nc):
        for f in nc.m.functions:
            for bb in f.blocks:
                il = bb.instructions
                i = 0
                while i < len(il):
                    inst = il[i]
                    si = inst.sync_info
                    if si is not None and len(si.on_wait) > 1:
                        waits = list(si.on_wait)
                        inst.sync_info = mybir.SyncInfo(
                            on_wait=[waits[-1]], on_update=list(si.on_update)
                        )
                        for w in waits[:-1]:
                            _uid[0] += 1
                            nop = mybir.InstEventSemaphore(
                                name=f"WSPLIT-{_uid[0]}",
                                engine=inst.engine,
                                ins=[],
                                outs=[],
                                sync_info=mybir.SyncInfo(
                                    on_wait=[w], on_update=[]),
                            )
                            il.insert(i, nop)
                            i += 1
                    i += 1

    def _drain_and_barrier(self, tick_clock, wait_clock):
        nc = self.nc
        drain_inst = nc.sync.drain()
        wait_clock.add_sem_waits(
            drain_inst.ins, ScopedClock({None: tick_clock.global_clock})
        )
        si = drain_inst.ins.sync_info
        if si is not None and len(si.on_wait) > 1:
            waits = list(si.on_wait)
            drain_inst.ins.sync_info = mybir.SyncInfo(
                on_wait=[waits[0]], on_update=list(si.on_update)
            )
            for w in waits[1:]:
                extra = nc.sync.drain()
                extra.ins.sync_info = mybir.SyncInfo(on_wait=[w], on_update=[])

        _split_multiwaits(nc)
        nc.all_engine_barrier()
        assert self.sems is not None
        popped = nc._tile_sem_poison_stack.pop()
        assert popped is self._sem_poison
        nc.clear_and_free_semaphores(list(self.sems.allocated().values()))
        nc.all_engine_barrier()

    tile.TileContext._drain_and_barrier = _drain_and_barrier
    _patched = True


def build_program():
    _patch_tile_drain()
    import concourse.bass as bass
    import concourse.tile as tile
    import concourse.mybir as mybir

    f32 = mybir.dt.float32
    bf16 = mybir.dt.bfloat16
    EXP = mybir.ActivationFunctionType.Exp

    nc = bass.Bass("TRN2", target_bir_lowering=False, debug=False,
                   num_devices=NCORES)

    xqT = nc.dram_tensor("xqT", [D, NT], bf16, kind="ExternalInput")
    xkT = nc.dram_tensor("xkT", [D, NT], bf16, kind="ExternalInput")
    xvT = nc.dram_tensor("xvT", [D, NT], bf16, kind="ExternalInput")
    wqT = nc.dram_tensor("wqT", [D, HGD], bf16, kind="ExternalInput")
    wkT = nc.dram_tensor("wkT", [D, HGD], bf16, kind="ExternalInput")
    wvT = nc.dram_tensor("wvT", [D, HGD], bf16, kind="ExternalInput")
    woT = nc.dram_tensor("woT", [HGD, D], bf16, kind="ExternalInput")
    out = nc.dram_tensor("out", [NQT, 128, D], f32, kind="ExternalOutput")

    partial = nc.dram_tensor("partial", [NT, D], bf16)
    rsout = nc.dram_tensor("rsout", [NQT, 128, D], bf16)

    with tile.TileContext(nc) as tc:
        from contextlib import ExitStack
        with ExitStack() as ctx:
            const = ctx.enter_context(tc.tile_pool(name="const", bufs=1))
            persist = ctx.enter_context(tc.tile_pool(name="persist", bufs=1))
            pt_pool = ctx.enter_context(tc.tile_pool(name="pt", bufs=3))
            outsb = ctx.enter_context(tc.tile_pool(name="outsb", bufs=2))
            # PSUM (8 banks of [128,2KB]): st = 2x[128,1024]f32 (4 banks,
            # scores + projection/out-proj accumulators), xa0/xa1 = PV
            # accumulators (2 banks), bc = broadcast reciprocals (1 bank).
            st_ps = ctx.enter_context(
                tc.tile_pool(name="st_ps", bufs=2, space="PSUM"))
            xa_ps = ctx.enter_context(
                tc.tile_pool(name="xa_ps", bufs=1, space="PSUM"))

            # --- constants + all input DMAs upfront ------------------------
            wq_sb = const.tile([128, 8, HGD], bf16)  # [k-part, k-tile, col]
            wk_sb = const.tile([128, 8, HGD], bf16)
            wv_sb = const.tile([128, 8, HGD], bf16)
            wo_sb = const.tile([128, 2, D], bf16)    # [d-part, hg k-tile, od]
            xk_st = persist.tile([128, 8, NT], bf16)
            xv_st = persist.tile([128, 8, NT], bf16)
            xq_st = persist.tile([128, 8, NT], bf16)
            # DMA issue order = consumption order; two DMAs per chunk so
            # the first projection matmul isn't gated on one 1MB transfer.
            nc.sync.dma_start(out=wk_sb[:], in_=wkT[:].rearrange(
                "(t p) c -> p t c", p=128))
            for st_, src in ((xk_st, xkT), (xv_st, xvT), (xq_st, xqT)):
                for n in range(4):
                    sl = slice(512 * n, 512 * (n + 1))
                    for h in range(2):
                        nc.sync.dma_start(
                            out=st_[:, 4 * h:4 * (h + 1), sl],
                            in_=src[512 * h:512 * (h + 1), sl]
                            .rearrange("(t p) c -> p t c", p=128))
                if st_ is xk_st:
                    nc.sync.dma_start(out=wv_sb[:], in_=wvT[:].rearrange(
                        "(t p) c -> p t c", p=128))
                elif st_ is xv_st:
                    nc.sync.dma_start(out=wq_sb[:], in_=wqT[:].rearrange(
                        "(t p) c -> p t c", p=128))
            nc.sync.dma_start(out=wo_sb[:], in_=woT[:].rearrange(
                "(t p) c -> p t c", p=128))

            # --- persistent activations -----------------------------------
            qt_sb = persist.tile([128, 2, NT], bf16)  # [qcol%128, qcol//128, tok]
            kt_sb = persist.tile([128, 2, NT], bf16)
            v_sb = persist.tile([128, NKT, 2 * VB], bf16)
            xt_sb = persist.tile([128, 2, NT], bf16)  # normalized x^T

            nc.vector.memset(v_sb[:], 1.0)

            # normalize scratch (persist; memset once so the stream
            # transposes never read uninitialized SBUF)
            xu0 = persist.tile([96, 512], f32)    # x(h even) + denom row 64
            xu1 = persist.tile([128, 512], f32)   # denom row 63, x rows 64+
            tr1a = persist.tile([96, 16, 32], f32)
            tr1b = persist.tile([64, 16, 32], f32)
            trba = persist.tile([96, 16, 32], bf16)
            trbb = persist.tile([64, 16, 32], bf16)
            tr3a = persist.tile([96, 512], bf16)
            tr3b = persist.tile([64, 512], bf16)
            ones_sb = const.tile([128, DH], bf16)
            nc.vector.memset(xu0[:], 1.0)
            nc.vector.memset(xu1[:], 1.0)
            nc.vector.memset(tr1a[:], 1.0)
            nc.vector.memset(tr1b[:], 1.0)
            nc.vector.memset(trba[:], 1.0)
            nc.vector.memset(trbb[:], 1.0)
            nc.vector.memset(ones_sb[:], 1.0)

            # --- Q/K projections ------------------------------------------
            def qk_proj(xst, wsb, dst):
                for n in range(NQT):
                    acc = st_ps.tile([128, 1024], f32, tag="st", name="qkacc")
                    for k in range(8):
                        for m in range(2):
                            nc.tensor.matmul(
                                acc[:, 512 * m:512 * (m + 1)],
                                wsb[:, k, 128 * m:128 * (m + 1)],
                                xst[:, k, 512 * n:512 * (n + 1)],
                                start=(k == 0), stop=(k == 7))
                    for m in range(2):
                        nc.vector.tensor_copy(
                            dst[:, m, 512 * n:512 * (n + 1)],
                            acc[:, 512 * m:512 * (m + 1)])

            qk_proj(xk_st, wk_sb, kt_sb)

            # --- V projection ---------------------------------------------
            for mg in range(4):
                acc = st_ps.tile([128, 1024], f32, tag="st", name="vacc")
                # m outer / k inner: two regions share a psum bank, so each
                # region's accumulation must complete before the next opens.
                for m in range(4):
                    for k in range(8):
                        nc.tensor.matmul(
                            acc[:, 256 * m:256 * m + HGD],
                            xv_st[:, k, 512 * mg + 128 * m:
                                  512 * mg + 128 * (m + 1)],
                            wv_sb[:, k, :],
                            start=(k == 0), stop=(k == 7))
                for m in range(4):
                    mt = 4 * mg + m
                    dstv = v_sb[:, mt, :].rearrange("p (g c) -> p g c", c=VB)
                    srcv = acc[:, 256 * m:256 * (m + 1)].rearrange(
                        "p (g c) -> p g c", c=128)
                    # heads {0,2} -> cols 0..63; heads {1,3} -> cols 65..128
                    nc.vector.tensor_copy(dstv[:, :, 0:DH],
                                          srcv[:, :, 0:DH])
                    nc.vector.tensor_copy(dstv[:, :, DH + 1:VB],
                                          srcv[:, :, DH:2 * DH])

            qk_proj(xq_st, wq_sb, qt_sb)

            # --- attention ------------------------------------------------
            partial_dmas = [[] for _ in range(NQT)]
            rs_ccs = []

            def att_hp(qt, hp, pending=None):
                """Emit one (q-tile, head-pair) attention block.  Returns a
                closure that finishes its normalization (PE broadcast +
                DVE multiplies); the caller threads it into the NEXT
                block's kt loop so the PE never stalls on the DVE chain."""
                qsl = slice(512 * qt, 512 * (qt + 1))
                xa0 = xa_ps.tile([DH + 1, 512], f32, tag="xa0")
                xa1 = xa_ps.tile([128, 512], f32, tag="xa1")

                def scores(kt):
                    st = st_ps.tile([128, 1024], f32, tag="st", name="st")
                    for j in range(2):
                        p0 = 64 * j
                        nc.tensor.matmul(
                            st[:, 512 * j:512 * (j + 1)],
                            kt_sb[p0:p0 + 64, hp,
                                  128 * kt:128 * (kt + 1)],
                            qt_sb[p0:p0 + 64, hp, qsl],
                            tile_position=(p0, 0))
                    return st

                def exp(st):
                    pt = pt_pool.tile([128, 1024], bf16)
                    nc.scalar.activation(pt[:], st[:], EXP, scale=SCALE)
                    return pt

                def pv(kt, pt):
                    nc.tensor.matmul(
                        xa0[:],
                        v_sb[:, kt, VB * hp:VB * hp + DH + 1],
                        pt[:, 0:512],
                        start=(kt == 0), stop=(kt == NKT - 1))
                    nc.tensor.matmul(
                        xa1[:],
                        v_sb[:, kt, VB * hp + 1:VB * hp + 129],
                        pt[:, 512:1024],
                        start=(kt == 0), stop=(kt == NKT - 1))

                pt_prev = exp(scores(0))
                for kt in range(1, NKT):
                    st = scores(kt)
                    if kt == 3 and pending is not None:
                        pending()
                    pv(kt - 1, pt_prev)
                    pt_prev = exp(st)
                pv(NKT - 1, pt_prev)

                # DVE-only denominator chain (no DMA -> immune to RS
                # traffic): free the PV psum, reshape denoms via stream
                # transpose, reciprocal at [32,16], transpose back.
                nc.vector.tensor_copy(xu0[0:DH + 1, :], xa0[:])
                nc.vector.tensor_copy(xu1[32:64, :], xa1[32:64, :])
                nc.vector.tensor_copy(xu1[64:128, :], xa1[64:128, :])
                f1a = tr1a[64:96, :, :].rearrange("p a b -> p (a b)")
                nc.vector.transpose(f1a, xu0[64:96, :])
                f1b = tr1b[32:64, :, :].rearrange("p a b -> p (a b)")
                nc.vector.transpose(f1b, xu1[32:64, :])
                # denom of j0 sits at window row 0 -> strided col 0;
                # denom of j1 sits at window row 31 -> strided col 31.
                nc.vector.reciprocal(tr1a[64:96, :, 0:1],
                                     tr1a[64:96, :, 0:1])
                nc.vector.reciprocal(tr1b[32:64, :, 31:32],
                                     tr1b[32:64, :, 31:32])
                nc.vector.tensor_copy(trba[64:96, :, 0:1],
                                      tr1a[64:96, :, 0:1])
                # write j1 recips at strided col 0 so they transpose back
                # to the 32-aligned row 32.
                nc.vector.tensor_copy(trbb[32:64, :, 0:1],
                                      tr1b[32:64, :, 31:32])
                nc.vector.transpose(
                    tr3a[64:96, :], trba[64:96, :, :]
                    .rearrange("p a b -> p (a b)"))
                nc.vector.transpose(
                    tr3b[32:64, :], trbb[32:64, :, :]
                    .rearrange("p a b -> p (a b)"))

                def finish():
                    bc = xa_ps.tile([128, 512], f32, tag="bc")
                    nc.tensor.matmul(bc[0:DH, :], ones_sb[64:65, :],
                                     tr3a[64:65, :], tile_position=(64, 0))
                    nc.tensor.matmul(bc[DH:128, :], ones_sb[32:33, :],
                                     tr3b[32:33, :], tile_position=(32, 64))
                    nc.vector.tensor_mul(
                        xt_sb[0:DH, hp, qsl], xu0[0:DH, :], bc[0:DH, :])
                    nc.vector.tensor_mul(
                        xt_sb[DH:128, hp, qsl], xu1[DH:128, :],
                        bc[DH:128, :])

                return finish

            def outproj_rs(qt):
                # out-proj: partial[t, o] = sum_d x^T[d, t] Wo^T[d, o]
                # (bias added on host).  qt 0-2: one 512-token RS chunk;
                # qt 3: two 256-token chunks to shorten the exposed tail.
                def op_step(acc, n, m):
                    osl = slice(512 * n, 512 * (n + 1))
                    tsl = slice(512 * qt + 128 * m,
                                512 * qt + 128 * (m + 1))
                    for k in range(2):
                        nc.tensor.matmul(
                            acc[:], xt_sb[:, k, tsl], wo_sb[:, k, osl],
                            start=(k == 0), stop=(k == 1))
                    ob = outsb.tile([128, 512], bf16, tag="ob")
                    nc.scalar.copy(ob[:], acc[:])
                    w = nc.sync.dma_start(out=partial[tsl, osl], in_=ob[:])
                    partial_dmas[qt].append(w)

                def emit_rs(tok0, ntok, out_ap):
                    cc = nc.gpsimd.collective_compute(
                        "ReduceScatter",
                        mybir.AluOpType.add,
                        replica_groups=GROUPS,
                        ins=[partial[tok0:tok0 + ntok, :]],
                        outs=[out_ap],
                    )
                    for w in partial_dmas[qt]:
                        tile.add_dep_helper(cc.ins, w.ins,
                                            reason="RS after partial")
                    rs_ccs.append(cc)

                if qt < NQT - 1:
                    steps = [(n, m) for n in range(2) for m in range(4)]
                else:
                    steps = [(n, m) for m in range(4) for n in range(2)]
                for i, (n, m) in enumerate(steps):
                    if i % 2 == 0:
                        stt = st_ps.tile([128, 1024], f32, tag="st",
                                         name="opacc")
                    op_step(stt[:, 512 * (i % 2):512 * (i % 2 + 1)], n, m)
                    if qt == NQT - 1 and i == 3:
                        emit_rs(512 * qt, 256, rsout[qt][0:64, :])
                if qt < NQT - 1:
                    emit_rs(512 * qt, 512, rsout[qt])
                else:
                    emit_rs(512 * qt + 256, 256, rsout[qt][64:128, :])

            # schedule: normalize finish of block i lands inside block
            # i+1's kt loop; out-proj/RS of tile qt-1 sits between the two
            # head-pair blocks of tile qt.
            fin = att_hp(0, 0)
            fin = att_hp(0, 1, pending=fin)
            for qt in range(1, NQT):
                fin2 = att_hp(qt, 0, pending=fin)
                outproj_rs(qt - 1)
                fin = att_hp(qt, 1, pending=fin2)
            fin()
            outproj_rs(NQT - 1)

            # tails at the very end: the in-order SP queue must never
            # head-block on a collective mid-kernel.
            for qt in range(NQT):
                rs_sb = outsb.tile([128, D], bf16, tag="rs_sb")
                ld = nc.sync.dma_start(out=rs_sb[:], in_=rsout[qt])
                for cc in rs_ccs:
                    tile.add_dep_helper(ld.ins, cc.ins,
                                        reason="load after RS")
                rs32 = outsb.tile([128, D], f32, tag="rs32")
                nc.vector.tensor_copy(rs32[:], rs_sb[:])
                nc.sync.dma_start(out=out[qt], in_=rs32[:])

    return nc


_CACHE = {}


def _get_program():
    if "nc" not in _CACHE:
        _CACHE["nc"] = build_program()
    return _CACHE["nc"]


def _bf16(x):
    import ml_dtypes
    return np.ascontiguousarray(np.asarray(x, dtype=ml_dtypes.bfloat16))


def make_in_maps(query, key, value, Wq, Wk, Wv, Wo, bo):
    """Host-side sharding: per-core input dicts."""
    query = np.asarray(query, dtype=np.float32)
    key = np.asarray(key, dtype=np.float32)
    value = np.asarray(value, dtype=np.float32)
    Wq = np.asarray(Wq, dtype=np.float32)
    Wk = np.asarray(Wk, dtype=np.float32)
    Wv = np.asarray(Wv, dtype=np.float32)
    Wo = np.asarray(Wo, dtype=np.float32)

    xT = [_bf16(x.T) for x in
          (query[0], key[0], value[0], query[1], key[1], value[1])]
    wq_c, wk_c, wv_c, wo_c = [], [], [], []
    for hg in range(CPB):
        hsl = slice(HGD * hg, HGD * (hg + 1))
        wq_c.append(_bf16(Wq[hsl, :].T))
        wk_c.append(_bf16(Wk[hsl, :].T))
        wv_c.append(_bf16(Wv[hsl, :].T))
        wo_c.append(_bf16(Wo[:, hsl].T))
    in_maps = []
    for c in range(NCORES):
        b, hg = divmod(c, CPB)
        in_maps.append({
            "xqT": xT[3 * b + 0],
            "xkT": xT[3 * b + 1],
            "xvT": xT[3 * b + 2],
            "wqT": wq_c[hg],
            "wkT": wk_c[hg],
            "wvT": wv_c[hg],
            "woT": wo_c[hg],
        })
    return in_maps


def assemble(results):
    """qt 0-2: core i owns strip 512*qt + 128*i.  qt 3 was reduce-
    scattered as two 256-token chunks: rows 0:64 of out[3] are tokens
    1536 + 64*i, rows 64:128 are tokens 1792 + 64*i."""
    out = np.empty((B, NT, D), dtype=np.float32)
    for c in range(NCORES):
        b, i = divmod(c, CPB)
        o = results[c]["out"]
        for qt in range(NQT - 1):
            s = 512 * qt + 128 * i
            out[b, s:s + 128, :] = o[qt]
        out[b, 1536 + 64 * i:1536 + 64 * (i + 1), :] = o[NQT - 1][0:64]
        out[b, 1792 + 64 * i:1792 + 64 * (i + 1), :] = o[NQT - 1][64:128]
    return out


def run(query, key, value, Wq, Wk, Wv, Wo, bo, trace=False):
    from concourse.bass_utils import run_bass_kernel_spmd
    nc = _get_program()
    in_maps = make_in_maps(query, key, value, Wq, Wk, Wv, Wo, bo)
    res = run_bass_kernel_spmd(nc, in_maps, core_ids=list(range(NCORES)),
                               trace=trace)
    out = assemble(res.results)
    out += np.asarray(bo, dtype=np.float32)
    return out, res


def kernel(query, key, value, qpos=None, kpos=None, Wq=None, Wk=None,
           Wv=None, Wo=None, bo=None):
    out, _ = run(query, key, value, Wq, Wk, Wv, Wo, bo)
    return out
